# revision 10
# baseline (speedup 1.0000x reference)
"""Causal attention kernel for Trainium2 (Bass/Tile), 8-core SPMD.

Problem: out = softmax(causal(C @ B^T / sqrt(64))) @ x
  x, B, C: [2, 8, 4096, 64] fp32  (V, K, Q respectively)

Sharding: batch*heads = 16 slices -> 2 per core across 8 cores.

Host-side prep (in kernel(), before launch): Q is scaled by log2(e)/8,
converted to bf16, transposed to [64, L] and duplicated onto both
partition halves ([128, L]); K is converted to bf16 and packed in
j-tile PAIRS ([d(j0)|d(j1), kk] per pair -> [128, L]); V is converted
to bf16. This makes every input DMA a contiguous-per-partition load
(full descriptor bandwidth) and removes all on-device staging compute.

Per-head device algorithm (L=4096, D=64, S^T score layout [kk, q]):
  - QK pairs are row-packed into the PE array (j0 -> rows 0-63, j1 ->
    rows 64-127 via tile_position): the two bf16 matmuls run
    concurrently, each writing its own PSUM bank; scores come out as
    y = log2(e)*(C.B)/8 so exp(s/8) == 2^y.
  - exp alternates per j-pair between ScalarE activation(Exp, scale=ln2)
    and a DVE "Schraudolph" exp2 (u16 = y*128 + B, bitcast bf16; max rel
    err ~3.3%), both reading the two PSUM banks in one instruction.
  - PV matmuls (bf16 probs x bf16 V+ones-column) accumulate O^T and the
    softmax denominator into one PSUM bank per 512-q block.
  - Causal masking: partial-width matmuls from each tile's first valid
    column + a triangular bf16 mask multiply on diagonal tiles (GpSimd).
  - Epilogue per 512-q block: ScalarE copies O^T to bf16 SBUF, 4 PE
    transposes into one PSUM bank, DVE reciprocal + broadcast multiply,
    one GpSimd(SWDGE)-issued DMA out.
"""

import os
from contextlib import ExitStack

import numpy as np

L = 4096
D = 64
P = 128
NT = L // P            # 32 q/k tiles per head
NJP = NT // 2          # 16 k-tile pairs
NQB = L // 512         # 8 512-col q blocks per head
HEADS_PER_CORE = 2
N_CORES = 8

LOG2E = 1.4426950408889634
QSCALE = LOG2E / 8.0   # folded into Q on the host
LN2 = 0.6931471805599453
SCH_MUL = 128.0        # bf16 exponent scale
SCH_BIAS = 16250.875   # 127*128 - 5.125 (tuned: max rel err 3.26%)

# exp engine split: DVE takes DVE_EXP_NUM of every DVE_EXP_DEN j-pairs.
DVE_EXP_NUM = int(os.environ.get("KERNEL_DVE_NUM", "4"))
DVE_EXP_DEN = int(os.environ.get("KERNEL_DVE_DEN", "9"))

NCH = 4                # input chunks per tensor (dep granularity)
TCH = NT // NCH        # 8 q/k tiles per chunk
JCH = NJP // NCH       # 4 jp per chunk

_cache = {}


def _build_nc():
    import concourse.mybir as mybir
    import concourse.tile as tile
    from concourse import bacc
    from concourse.masks import make_identity

    f32 = mybir.dt.float32
    bf16 = mybir.dt.bfloat16
    u16 = mybir.dt.uint16
    EXP = mybir.ActivationFunctionType.Exp

    nc = bacc.Bacc("TRN2", target_bir_lowering=False, debug=False)

    qt_t = nc.dram_tensor("qt", (HEADS_PER_CORE, P, L), bf16, kind="ExternalInput")
    kt_t = nc.dram_tensor("kt", (HEADS_PER_CORE, P, NJP * P), bf16, kind="ExternalInput")
    xb_t = nc.dram_tensor("xb", (HEADS_PER_CORE, L, D), bf16, kind="ExternalInput")
    o_t = nc.dram_tensor("out", (HEADS_PER_CORE, L, D), f32, kind="ExternalOutput")
    qt_ap, kt_ap, xb_ap, o_ap = qt_t.ap(), kt_t.ap(), xb_t.ap(), o_t.ap()

    with tile.TileContext(nc) as tc, ExitStack() as ctx:
        const = ctx.enter_context(tc.tile_pool(name="const", bufs=1))
        ident = const.tile([P, P], bf16)
        make_identity(nc, ident[:])
        # Diagonal-tile mask in S^T coords [kk, q]: keep kk <= q.
        dmask = const.tile([P, P], bf16)
        nc.gpsimd.memset(dmask[:], 1.0)
        nc.gpsimd.affine_select(
            out=dmask[:],
            in_=dmask[:],
            compare_op=mybir.AluOpType.is_ge,
            fill=0.0,
            base=0,
            pattern=[[1, P]],       # +q
            channel_multiplier=-1,  # -kk  => keep where q - kk >= 0
        )

        qkv = ctx.enter_context(tc.tile_pool(name="qkv", bufs=1))
        qt2, kt2, v1 = {}, {}, {}
        for h in range(HEADS_PER_CORE):
            qt2[h] = [
                qkv.tile([P, TCH * P], bf16, name=f"qt2_{h}_{c}", tag=f"qt2_{h}_{c}")
                for c in range(NCH)
            ]
            kt2[h] = [
                qkv.tile([P, JCH, P], bf16, name=f"kt2_{h}_{c}", tag=f"kt2_{h}_{c}")
                for c in range(NCH)
            ]
            v1[h] = [
                qkv.tile([P, TCH, D + 1], bf16, name=f"v1_{h}_{c}", tag=f"v1_{h}_{c}")
                for c in range(NCH)
            ]

        for h in range(HEADS_PER_CORE):
            for cs in range(NCH):
                nc.sync.dma_start(
                    out=qt2[h][cs][:],
                    in_=qt_ap[h, :, cs * TCH * P : (cs + 1) * TCH * P],
                )
                nc.sync.dma_start(
                    out=kt2[h][cs][:],
                    in_=kt_ap[h, :, cs * JCH * P : (cs + 1) * JCH * P].rearrange(
                        "p (a b) -> p a b", b=P
                    ),
                )
                r = slice(cs * TCH * P, (cs + 1) * TCH * P)
                nc.sync.dma_start(
                    out=v1[h][cs][:, :, 0:D],
                    in_=xb_ap[h, r].rearrange("(j p) d -> p j d", p=P),
                )
                nc.gpsimd.memset(v1[h][cs][:, :, D], 1.0)

        # ---- Attention ----
        score_pool = ctx.enter_context(tc.tile_pool(name="score", bufs=2, space="PSUM"))
        oacc_pool = ctx.enter_context(tc.tile_pool(name="oacc", bufs=2, space="PSUM"))
        epips_pool = ctx.enter_context(tc.tile_pool(name="epips", bufs=2, space="PSUM"))
        ets_pool = ctx.enter_context(tc.tile_pool(name="ets", bufs=4))
        epi_pool = ctx.enter_context(tc.tile_pool(name="epi", bufs=2))

        expctr = 0
        for h in range(HEADS_PER_CORE):
            for qb in range(NQB):
                q0 = qb * 512
                jtop = (q0 + 511) // P          # last valid j tile (always odd)
                last_jp = jtop // 2
                ob = oacc_pool.tile([D + 1, 512], f32, name="ob", tag="ob")
                for jp in range(last_jp + 1):
                    j0, j1 = 2 * jp, 2 * jp + 1
                    v0 = max(0, j0 * P - q0)
                    v1_ = max(0, j1 * P - q0)

                    sc = score_pool.tile([P, 2, 512], f32, name="sc", tag="sc")
                    et = ets_pool.tile([P, 2, 512], bf16, name="et", tag="et")

                    qc = qt2[h][q0 // 1024]
                    qo = q0 % 1024
                    kc = kt2[h][jp // JCH]
                    jpo = jp % JCH
                    nc.tensor.matmul(
                        sc[:, 0, v0:512],
                        lhsT=kc[0:D, jpo],
                        rhs=qc[0:D, qo + v0 : qo + 512],
                        start=True, stop=True,
                    )
                    nc.tensor.matmul(
                        sc[:, 1, v1_:512],
                        lhsT=kc[D:P, jpo],
                        rhs=qc[D:P, qo + v1_ : qo + 512],
                        start=True, stop=True,
                        tile_position=(D, 0),
                    )

                    use_dve = (expctr * DVE_EXP_NUM) % DVE_EXP_DEN < DVE_EXP_NUM
                    expctr += 1

                    def _exp(dst, src):
                        if use_dve:
                            nc.vector.tensor_scalar(
                                out=dst.bitcast(u16), in0=src,
                                scalar1=SCH_MUL, scalar2=SCH_BIAS,
                                op0=mybir.AluOpType.mult,
                                op1=mybir.AluOpType.add,
                            )
                        else:
                            nc.scalar.activation(dst, src, EXP, scale=LN2)

                    if v0 == 0 and v1_ == 0:
                        _exp(
                            et[:].rearrange("p a n -> p (a n)"),
                            sc[:].rearrange("p a n -> p (a n)"),
                        )
                    else:
                        _exp(et[:, 0, v0:512], sc[:, 0, v0:512])
                        _exp(et[:, 1, v1_:512], sc[:, 1, v1_:512])

                    if j0 * P >= q0:
                        nc.gpsimd.tensor_mul(
                            et[:, 0, v0 : v0 + P],
                            et[:, 0, v0 : v0 + P],
                            dmask[:],
                        )
                    if j1 * P >= q0:
                        nc.gpsimd.tensor_mul(
                            et[:, 1, v1_ : v1_ + P],
                            et[:, 1, v1_ : v1_ + P],
                            dmask[:],
                        )

                    nc.tensor.matmul(
                        ob[:, v0:512],
                        lhsT=v1[h][j0 // TCH][:, j0 % TCH],
                        rhs=et[:, 0, v0:512],
                        start=(jp == 0),
                        stop=False,
                    )
                    nc.tensor.matmul(
                        ob[:, v1_:512],
                        lhsT=v1[h][j1 // TCH][:, j1 % TCH],
                        rhs=et[:, 1, v1_:512],
                        start=False,
                        stop=(jp == last_jp),
                    )

                # ---- Epilogue for this 512-q block ----
                osb = epi_pool.tile([D + 1, 512], bf16, name="osb", tag="osb")
                nc.scalar.copy(osb[:], ob[:])
                tp = epips_pool.tile([P, 4, D + 2], bf16, name="tp", tag="tp")
                for a in range(4):
                    nc.tensor.transpose(
                        tp[:, a, 0 : D + 1],
                        osb[:, a * P : (a + 1) * P],
                        ident[0 : D + 1, 0 : D + 1],
                    )
                rec = epi_pool.tile([P, 4], f32, name="rec", tag="rec")
                nc.vector.reciprocal(rec[:], tp[:, :, D])
                outsb = epi_pool.tile([P, 4, D], f32, name="outsb", tag="outsb")
                nc.vector.tensor_tensor(
                    out=outsb[:],
                    in0=tp[:, :, 0:D],
                    in1=rec[:].unsqueeze(2).broadcast_to((P, 4, D)),
                    op=mybir.AluOpType.mult,
                )
                nc.gpsimd.dma_start(
                    out=o_ap[h, q0 : q0 + 512].rearrange("(a p) d -> p a d", p=P),
                    in_=outsb[:],
                )

    nc.compile()
    return nc


def _get_nc():
    if "nc" not in _cache:
        _cache["nc"] = _build_nc()
    return _cache["nc"]


def _prep_inputs(x, B, C):
    """Host-side layout prep. Returns per-(batch*head) arrays:
    qt [128, L] bf16 (Q^T*scale duplicated on both halves),
    kt [128, L] bf16 (K^T packed j-pairs), xb [L, D] bf16."""
    import ml_dtypes

    bf = ml_dtypes.bfloat16
    nbh = x.shape[0]
    qs = (C.astype(np.float32) * np.float32(QSCALE)).astype(bf)   # [nbh, L, D]
    qtd = np.concatenate([qs.transpose(0, 2, 1)] * 2, axis=1)     # [nbh, 128, L]
    kb = B.astype(np.float32).astype(bf)
    kr = kb.reshape(nbh, NJP, 2, P, D)
    ktp = np.ascontiguousarray(kr.transpose(0, 2, 4, 1, 3)).reshape(nbh, P, NJP * P)
    xb = x.astype(np.float32).astype(bf)
    return np.ascontiguousarray(qtd), ktp, np.ascontiguousarray(xb)


def kernel(x: np.ndarray, B: np.ndarray, C: np.ndarray) -> np.ndarray:
    from concourse import bass_utils

    BATCH, H = x.shape[0], x.shape[1]
    nbh = BATCH * H
    xf = x.reshape(nbh, L, D)
    bf_ = B.reshape(nbh, L, D)
    cf = C.reshape(nbh, L, D)
    qtd, ktp, xb = _prep_inputs(xf, bf_, cf)

    nc = _get_nc()
    in_maps = []
    for c in range(N_CORES):
        s = slice(c * HEADS_PER_CORE, (c + 1) * HEADS_PER_CORE)
        in_maps.append(
            {
                "qt": np.ascontiguousarray(qtd[s]),
                "kt": np.ascontiguousarray(ktp[s]),
                "xb": np.ascontiguousarray(xb[s]),
            }
        )

    trace = bool(int(os.environ.get("KERNEL_TRACE", "0")))
    res = bass_utils.run_bass_kernel_spmd(
        nc,
        in_maps,
        core_ids=list(range(N_CORES)),
        trace=trace,
        trace_cores=list(range(N_CORES)) if trace else None,
    )
    _cache["last_result"] = res

    out = np.empty((nbh, L, D), dtype=np.float32)
    for c in range(N_CORES):
        out[c * HEADS_PER_CORE : (c + 1) * HEADS_PER_CORE] = res.results[c]["out"]
    return out.reshape(BATCH, H, L, D)
